# revision 11
# baseline (speedup 1.0000x reference)
"""Trainium2 Bass kernel for nn_CustomConvLayer (bilinear-tap conv).

Math: each of the K=9 taps gathers x at constant sub-pixel offset
(dy, dx) via separable bilinear interpolation, scales by a per-
(cout, cin, tap) weight, and accumulates over taps and input channels.
Because the offsets are constant over the spatial grid, each tap is a
fixed 2x2 blend of integer-shifted copies of the padded input, so the
whole op folds exactly into a dense (Fy x Fx) convolution (Fy, Fx <= 5;
4x4 for the reference offsets) with host-folded weights:

    W_eff[o, i, u, v] = sum_k w[o, i, k] * cy_k(u) * cx_k(v)
    out[b, o, y, x]   = sum_{i,u,v} W_eff[o,i,u,v] * xp[b, i, y+u, x+v]

where xp is x zero-padded by 1 on top/left and >=Fy-1 on bottom/right
(the reference's index clipping lands in the zero-pad rows, so it is
exactly a zero-padded conv).

Distribution: data-parallel over batch, one image per NeuronCore (B=8,
8 cores). Per core the conv runs as TensorE matmuls:
  - SBUF x layout: even padded rows on partitions 0-63 (64 channels),
    odd rows on partitions 64-127; 66 row-pair segments x 132 padded
    cols -> contraction K = 128 = (2 rows x 64 cin).
  - M = 128 = (2 output rows x 64 cout), N = 512 = (4 output row-pairs
    x W=128) per matmul, PSUM-accumulated over the 12 (delta, v) weight
    blocks of the banded row-pair structure.
"""

import os

import numpy as np
import ml_dtypes

import concourse.bass as bass  # noqa: F401  (bass types reachable via tile/bacc)
import concourse.mybir as mybir
import concourse.tile as tile
from concourse import bacc
from concourse.bass_utils import run_bass_kernel_spmd

B, CIN, H, W = 8, 64, 128, 128
COUT, KTAPS = 64, 9
NCORES = 8
WP = 132          # padded segment width (xp cols 0..131; 0,129.. are zero)
NSEG = 66         # row-pair segments (xp rows 2t, 2t+1 for t in 0..65)
NSG = 16          # supergroups of 8 output rows

DTYPE_TAG = os.environ.get("CONV_DTYPE", "f32r")  # f32r | bf16 | f32
TRACE = bool(int(os.environ.get("CONV_TRACE", "0")))

_MYBIR_DT = {
    "bf16": mybir.dt.bfloat16,
    "f32r": mybir.dt.float32r,
    "f32": mybir.dt.float32,
}
_NP_DT = {
    "bf16": ml_dtypes.bfloat16,
    "f32r": np.float32,
    "f32": np.float32,
}


def fold_weights(weights, tap_offsets):
    """Fold per-tap scalar weights + bilinear coeffs into W_eff
    [COUT, CIN, Fy, Fx] (float64)."""
    w = np.asarray(weights, np.float64)
    off = np.asarray(tap_offsets, np.float64)
    dy, dx = off[:, 0], off[:, 1]
    assert (dy >= 0).all() and (dx >= 0).all(), "negative tap offsets unsupported"
    iy = np.floor(dy).astype(np.int64)
    fy = dy - iy
    ix = np.floor(dx).astype(np.int64)
    fx = dx - ix
    Fy = int(iy.max()) + 2
    Fx = int(ix.max()) + 2
    assert Fy <= 5 and Fx <= 5
    Weff = np.zeros((COUT, CIN, Fy, Fx))
    for k in range(KTAPS):
        for a, cy in ((0, 1.0 - fy[k]), (1, fy[k])):
            for bb, cx in ((0, 1.0 - fx[k]), (1, fx[k])):
                Weff[:, :, iy[k] + a, ix[k] + bb] += w[:, :, k] * (cy * cx)
    return Weff


def make_blocks(Weff):
    """Build the (delta, v) lhsT blocks of the banded row-pair matmul
    structure.

    Block (delta, v) couples input row-pair t = 4*sg + g + delta to
    output row-pair 4*sg + g:  lhsT[(j, ci), (i, co)] = W_eff[co, ci,
    u = 2*delta + j - i, v] (zero when u out of range).

    Returns (blocks, Wh) where blocks is a list of (delta, v) and Wh is
    [128, nblk*128] float32 with block bi at columns bi*128:(bi+1)*128.
    """
    _, _, Fy, Fx = Weff.shape
    ndelta = Fy // 2 + 1
    blocks, mats = [], []
    for d in range(ndelta):
        for v in range(Fx):
            Mb = np.zeros((128, 128))
            nz = False
            for j in (0, 1):
                for i in (0, 1):
                    u = 2 * d + j - i
                    if 0 <= u < Fy:
                        blk = Weff[:, :, u, v].T  # [cin, cout]
                        Mb[j * 64:(j + 1) * 64, i * 64:(i + 1) * 64] = blk
                        nz = nz or bool(np.abs(blk).max() > 0)
            if nz:
                blocks.append((d, v))
                mats.append(Mb)
    Wh = np.stack(mats, 0).transpose(1, 0, 2).reshape(128, -1)
    return blocks, np.ascontiguousarray(Wh, dtype=np.float64)


def build(blocks, dtag, repeat=1):
    """Build + compile the per-core Bass module (SPMD; same program on
    every core, per-core inputs). `repeat` re-emits the whole body N
    times (for wall-clock-delta benchmarking only)."""
    # f32r: the BIR verifier requires matmul producers typed float32r,
    # but walrus rejects float32r memsets — so tiles/DRAM are float32r
    # and only the (zero-valued) memsets bitcast to plain float32.
    dt_x = _MYBIR_DT[dtag]
    ms_cast = (lambda ap: ap.bitcast(mybir.dt.float32)) if dtag == "f32r" else (
        lambda ap: ap
    )
    nblk = len(blocks)
    nc = bacc.Bacc(
        "TRN2", target_bir_lowering=False, debug=False, enable_asserts=False
    )
    xs = nc.dram_tensor("xs", [CIN, H, W], dt_x, kind="ExternalInput")
    wb = nc.dram_tensor("wb", [128, nblk * 128], dt_x, kind="ExternalInput")
    out = nc.dram_tensor("out", [COUT, H, W], mybir.dt.float32, kind="ExternalOutput")

    with tile.TileContext(nc) as tc:
        with (
            tc.tile_pool(name="const", bufs=1) as const_pool,
            tc.tile_pool(name="psum", bufs=8, space="PSUM") as psum_pool,
            tc.tile_pool(name="stage", bufs=8) as stage_pool,
        ):
            xbuf = const_pool.tile([128, NSEG * WP], dt_x, tag="xbuf")
            wbuf = const_pool.tile([128, nblk * 128], dt_x, tag="wbuf")
            xv = xbuf[:].rearrange("p (t c) -> p t c", c=WP)

            for _rep in range(repeat):
                # ---- weights ----
                nc.sync.dma_start(out=wbuf[:], in_=wb.ap())

                # ---- zero padding regions of the x buffer ----
                nc.gpsimd.memset(ms_cast(xv[:, :, 0:1]), 0.0)        # left pad
                nc.gpsimd.memset(ms_cast(xv[:, :, 129:132]), 0.0)    # right pad
                nc.gpsimd.memset(ms_cast(xv[0:64, 0:1, :]), 0.0)     # xp row 0
                nc.gpsimd.memset(ms_cast(xv[64:128, 64:65, :]), 0.0)  # xp row 129
                nc.gpsimd.memset(ms_cast(xv[:, 65:66, :]), 0.0)      # xp 130/131

                # ---- x loads: xp row r = x row r-1; partition (j, ci) of
                # segment t holds xp row 2t+j. Bulk covers t=1..63 (x rows
                # 1..126); edges: xp row 1 (x row 0) and xp row 128 (x 127).
                xap = xs.ap()
                # chunked bulk load so matmuls start before the tail lands
                for t0, t1 in ((1, 17), (17, 33), (33, 49), (49, 64)):
                    dram = xap[:, 2 * t0 - 1:2 * t1 - 1, :].rearrange(
                        "ci (t j) x -> j ci t x", j=2
                    )
                    for j in (0, 1):
                        nc.sync.dma_start(
                            out=xv[j * 64:(j + 1) * 64, t0:t1, 1:129], in_=dram[j]
                        )
                nc.sync.dma_start(out=xv[64:128, 0:1, 1:129], in_=xap[:, 0:1, :])
                nc.sync.dma_start(out=xv[0:64, 64:65, 1:129], in_=xap[:, 127:128, :])

                # out[co, 8s + 2g + i, x] viewed as [s][i, co, g, x]
                out_ap = out.ap().rearrange("co (s g i) x -> s i co g x", g=4, i=2)

                # ---- matmuls: 2 halves x 8 supergroups x nblk blocks ----
                for half in range(2):
                    ptiles = [
                        psum_pool.tile(
                            [128, 4, 128], mybir.dt.float32, tag="ps",
                            name=f"ps_{_rep}_{half}_{k}",
                        )
                        for k in range(8)
                    ]
                    for bi, (d, v) in enumerate(blocks):
                        lhsT = wbuf[:, bi * 128:(bi + 1) * 128]
                        first = bi == 0
                        last = bi == nblk - 1
                        for sg in range(half * 8, half * 8 + 8):
                            t0 = 4 * sg + d
                            rhs = xv[:, t0:t0 + 4, v:v + 128]
                            nc.tensor.matmul(
                                ptiles[sg % 8][:], lhsT, rhs,
                                start=first, stop=last,
                            )
                    for sg in range(half * 8, half * 8 + 8):
                        st = stage_pool.tile(
                            [128, 4, 128], mybir.dt.float32, tag="st",
                            name=f"st_{_rep}_{half}_{sg}",
                        )
                        nc.vector.tensor_copy(st[:], ptiles[sg % 8][:])
                        for i in (0, 1):
                            nc.sync.dma_start(
                                out=out_ap[sg][i], in_=st[i * 64:(i + 1) * 64]
                            )

    nc.compile()
    return nc


_CACHE = {}


def _get_nc(blocks, dtag, repeat=1):
    key = (tuple(blocks), dtag, repeat)
    if key not in _CACHE:
        _CACHE[key] = build(blocks, dtag, repeat)
    return _CACHE[key]


def kernel(x, weights, tap_offsets):
    x = np.asarray(x)
    weights = np.asarray(weights)
    tap_offsets = np.asarray(tap_offsets)
    assert x.shape == (B, CIN, H, W)

    Weff = fold_weights(weights, tap_offsets)
    blocks, Wh = make_blocks(Weff)
    np_dt = _NP_DT[DTYPE_TAG]
    nc = _get_nc(blocks, DTYPE_TAG)

    Whc = np.ascontiguousarray(Wh.astype(np_dt))
    in_maps = [
        {"xs": np.ascontiguousarray(x[b].astype(np_dt)), "wb": Whc}
        for b in range(B)
    ]
    res = run_bass_kernel_spmd(nc, in_maps, list(range(NCORES)), trace=False)
    outs = [res.results[c]["out"] for c in range(NCORES)]
    return np.stack(outs, 0).astype(np.float32)


# revision 15
# speedup vs baseline: 3.1618x; 3.1618x over previous
"""Trainium2 Bass kernel for nn_CustomConvLayer (bilinear-tap conv).

Math: each of the K=9 taps gathers x at constant sub-pixel offset
(dy, dx) via separable bilinear interpolation, scales by a per-
(cout, cin, tap) weight, and accumulates over taps and input channels.
Because the offsets are constant over the spatial grid, each tap is a
fixed 2x2 blend of integer-shifted copies of the padded input, so the
whole op folds exactly into a dense (Fy x Fx) convolution (Fy, Fx <= 5;
4x4 for the reference offsets) with host-folded weights:

    W_eff[o, i, u, v] = sum_k w[o, i, k] * cy_k(u) * cx_k(v)
    out[b, o, y, x]   = sum_{i,u,v} W_eff[o,i,u,v] * xp[b, i, y+u, x+v]

where xp is x zero-padded by 1 on top/left and >=Fy-1 on bottom/right
(the reference's index clipping lands in the zero-pad rows, so it is
exactly a zero-padded conv).

Distribution: data-parallel over batch, one image per NeuronCore (B=8,
8 cores). Per core the conv runs as TensorE matmuls:
  - SBUF x layout: even padded rows on partitions 0-63 (64 channels),
    odd rows on partitions 64-127; 66 row-pair segments x 132 padded
    cols -> contraction K = 128 = (2 rows x 64 cin).
  - M = 128 = (2 output rows x 64 cout), N = 512 = (4 output row-pairs
    x W=128) per matmul, PSUM-accumulated over the 12 (delta, v) weight
    blocks of the banded row-pair structure.
"""

import os

import numpy as np
import ml_dtypes

import concourse.bass as bass  # noqa: F401  (bass types reachable via tile/bacc)
import concourse.mybir as mybir
import concourse.tile as tile
from concourse import bacc
from concourse.bass_utils import run_bass_kernel_spmd

B, CIN, H, W = 8, 64, 128, 128
COUT, KTAPS = 64, 9
NCORES = 8
WP = 132          # padded segment width (xp cols 0..131; 0,129.. are zero)
NSEG = 66         # row-pair segments (xp rows 2t, 2t+1 for t in 0..65)
NSG = 16          # supergroups of 8 output rows

DTYPE_TAG = os.environ.get("CONV_DTYPE", "f32r")  # f32r | bf16 | f32
PLAN = int(os.environ.get("CONV_PLAN", "1"))

_MYBIR_DT = {
    "bf16": mybir.dt.bfloat16,
    "f32r": mybir.dt.float32r,
    "f32": mybir.dt.float32,
}
_NP_DT = {
    "bf16": ml_dtypes.bfloat16,
    "f32r": np.float32,
    "f32": np.float32,
}


def fold_weights(weights, tap_offsets):
    """Fold per-tap scalar weights + bilinear coeffs into W_eff
    [COUT, CIN, Fy, Fx] (float64)."""
    w = np.asarray(weights, np.float64)
    off = np.asarray(tap_offsets, np.float64)
    dy, dx = off[:, 0], off[:, 1]
    assert (dy >= 0).all() and (dx >= 0).all(), "negative tap offsets unsupported"
    iy = np.floor(dy).astype(np.int64)
    fy = dy - iy
    ix = np.floor(dx).astype(np.int64)
    fx = dx - ix
    Fy = int(iy.max()) + 2
    Fx = int(ix.max()) + 2
    assert Fy <= 5 and Fx <= 5
    Weff = np.zeros((COUT, CIN, Fy, Fx))
    for k in range(KTAPS):
        for a, cy in ((0, 1.0 - fy[k]), (1, fy[k])):
            for bb, cx in ((0, 1.0 - fx[k]), (1, fx[k])):
                Weff[:, :, iy[k] + a, ix[k] + bb] += w[:, :, k] * (cy * cx)
    return Weff


def make_blocks(Weff):
    """Build the (delta, v) lhsT blocks of the banded row-pair matmul
    structure.

    Block (delta, v) couples input row-pair t = 4*sg + g + delta to
    output row-pair 4*sg + g:  lhsT[(j, ci), (i, co)] = W_eff[co, ci,
    u = 2*delta + j - i, v] (zero when u out of range).

    Returns (blocks, Wh) where blocks is a list of (delta, v) and Wh is
    [128, nblk*128] float32 with block bi at columns bi*128:(bi+1)*128.
    """
    _, _, Fy, Fx = Weff.shape
    ndelta = Fy // 2 + 1
    blocks, mats = [], []
    for d in range(ndelta):
        for v in range(Fx):
            Mb = np.zeros((128, 128))
            nz = False
            for j in (0, 1):
                for i in (0, 1):
                    u = 2 * d + j - i
                    if 0 <= u < Fy:
                        blk = Weff[:, :, u, v].T  # [cin, cout]
                        Mb[j * 64:(j + 1) * 64, i * 64:(i + 1) * 64] = blk
                        nz = nz or bool(np.abs(blk).max() > 0)
            if nz:
                blocks.append((d, v))
                mats.append(Mb)
    Wh = np.stack(mats, 0).transpose(1, 0, 2).reshape(128, -1)
    return blocks, np.ascontiguousarray(Wh, dtype=np.float64)


def make_quadrant_plan(Weff):
    """Plan-2: decompose into 64x64 quadrant matmuls packed 4-at-a-time
    into the PE array via tile_position.

    Each (i, u, v) contribution is a K=64 (cin), M=64 (cout) matmul:
      out row 2s+i  +=  Weff[:, :, u, v].T @ xp[row 2s+i+u, col +v]
    xp row parity = (i+u)%2 selects the SBUF partition half (kh); the
    output row parity i selects the PSUM partition half (mh). Rounds of
    4 with distinct (kh, mh) run concurrently on the array.

    Returns (rounds, Wh2): rounds is a list of lists of
    (slot, kh, mh, toff, v); Wh2 is [128, nslots*64] float64.
    """
    _, _, Fy, Fx = Weff.shape
    quads = {}
    for i in (0, 1):
        for u in range(Fy):
            quads.setdefault(((i + u) % 2, i), []).append((i, u))
    depth = max(len(v) for v in quads.values())
    rounds, mats = [], []
    nslot = 0
    for v in range(Fx):
        for r in range(depth):
            members = []  # (kh, mh, i, u)
            for (kh, mh), lst in sorted(quads.items()):
                if r < len(lst):
                    i, u = lst[r]
                    members.append((kh, mh, i, u))
            # pack kh=0 and kh=1 blocks pairwise into column slots
            kh0 = [m for m in members if m[0] == 0]
            kh1 = [m for m in members if m[0] == 1]
            nsl = max(len(kh0), len(kh1))
            rnd = []
            for s in range(nsl):
                slot = nslot + s
                for lst2 in (kh0, kh1):
                    if s < len(lst2):
                        kh, mh, i, u = lst2[s]
                        rnd.append((slot, kh, mh, (i + u) // 2, v))
                        mats.append((slot, kh, Weff[:, :, u, v].T))
            nslot += nsl
            rounds.append(rnd)
    Wh2 = np.zeros((128, nslot * 64))
    for slot, kh, blk in mats:
        Wh2[kh * 64:(kh + 1) * 64, slot * 64:(slot + 1) * 64] = blk
    return rounds, Wh2


def build(blocks, dtag, repeat=1, plan=1):
    """Build + compile the per-core Bass module (SPMD; same program on
    every core, per-core inputs). `repeat` re-emits the whole body N
    times (for wall-clock-delta benchmarking only). plan=1: 128x128
    banded blocks; plan=2: `blocks` is the quadrant rounds list."""
    # f32r: the BIR verifier requires matmul producers typed float32r,
    # but walrus rejects float32r memsets — so tiles/DRAM are float32r
    # and only the (zero-valued) memsets bitcast to plain float32.
    dt_x = _MYBIR_DT[dtag]
    ms_cast = (lambda ap: ap.bitcast(mybir.dt.float32)) if dtag == "f32r" else (
        lambda ap: ap
    )
    nblk = len(blocks)
    if plan == 1:
        wcols = nblk * 128
    else:
        wcols = 64 * (1 + max(m[0] for rnd in blocks for m in rnd))
    nc = bacc.Bacc(
        "TRN2", target_bir_lowering=False, debug=False, enable_asserts=False
    )
    xs = nc.dram_tensor("xs", [CIN, H, W], dt_x, kind="ExternalInput")
    wb = nc.dram_tensor("wb", [128, wcols], dt_x, kind="ExternalInput")
    out = nc.dram_tensor("out", [COUT, H, W], mybir.dt.float32, kind="ExternalOutput")

    with tile.TileContext(nc) as tc:
        with (
            tc.tile_pool(name="const", bufs=1) as const_pool,
            tc.tile_pool(name="psum", bufs=8, space="PSUM") as psum_pool,
            tc.tile_pool(name="stage", bufs=8) as stage_pool,
        ):
            xbuf = const_pool.tile([128, NSEG * WP], dt_x, tag="xbuf")
            wbuf = const_pool.tile([128, wcols], dt_x, tag="wbuf")
            xv = xbuf[:].rearrange("p (t c) -> p t c", c=WP)

            for _rep in range(repeat):
                # ---- weights ----
                nc.sync.dma_start(out=wbuf[:], in_=wb.ap())

                # ---- zero padding regions of the x buffer ----
                nc.gpsimd.memset(ms_cast(xv[:, :, 0:1]), 0.0)        # left pad
                nc.gpsimd.memset(ms_cast(xv[:, :, 129:132]), 0.0)    # right pad
                nc.gpsimd.memset(ms_cast(xv[0:64, 0:1, :]), 0.0)     # xp row 0
                nc.gpsimd.memset(ms_cast(xv[64:128, 64:65, :]), 0.0)  # xp row 129
                nc.gpsimd.memset(ms_cast(xv[:, 65:66, :]), 0.0)      # xp 130/131

                # ---- x loads: xp row r = x row r-1; partition (j, ci) of
                # segment t holds xp row 2t+j. Bulk covers t=1..63 (x rows
                # 1..126); edges: xp row 1 (x row 0) and xp row 128 (x 127).
                xap = xs.ap()
                # chunked bulk load so matmuls start before the tail lands
                for t0, t1 in ((1, 17), (17, 33), (33, 49), (49, 64)):
                    dram = xap[:, 2 * t0 - 1:2 * t1 - 1, :].rearrange(
                        "ci (t j) x -> j ci t x", j=2
                    )
                    for j in (0, 1):
                        nc.sync.dma_start(
                            out=xv[j * 64:(j + 1) * 64, t0:t1, 1:129], in_=dram[j]
                        )
                nc.sync.dma_start(out=xv[64:128, 0:1, 1:129], in_=xap[:, 0:1, :])
                nc.sync.dma_start(out=xv[0:64, 64:65, 1:129], in_=xap[:, 127:128, :])

                # out[co, 8s + 2g + i, x] viewed as [s][i, co, g, x]
                out_ap = out.ap().rearrange("co (s g i) x -> s i co g x", g=4, i=2)

                # ---- matmuls: 2 halves x 8 supergroups ----
                for half in range(2):
                    ptiles = [
                        psum_pool.tile(
                            [128, 4, 128], mybir.dt.float32, tag="ps",
                            name=f"ps_{_rep}_{half}_{k}",
                        )
                        for k in range(8)
                    ]
                    if plan == 1:
                        for bi, (d, v) in enumerate(blocks):
                            lhsT = wbuf[:, bi * 128:(bi + 1) * 128]
                            first = bi == 0
                            last = bi == nblk - 1
                            for sg in range(half * 8, half * 8 + 8):
                                t0 = 4 * sg + d
                                rhs = xv[:, t0:t0 + 4, v:v + 128]
                                nc.tensor.matmul(
                                    ptiles[sg % 8][:], lhsT, rhs,
                                    start=first, stop=last,
                                )
                    else:
                        # quadrant packing: 4 concurrent 64x64 matmuls
                        nmm = {}  # (sg, mh) -> count, for start/stop
                        for rnd in blocks:
                            for _, _, mh, _, _ in rnd:
                                for sg in range(half * 8, half * 8 + 8):
                                    nmm[(sg, mh)] = nmm.get((sg, mh), 0) + 1
                        seen = {}
                        for rnd in blocks:
                            for sg in range(half * 8, half * 8 + 8):
                                for slot, kh, mh, toff, v in rnd:
                                    k = (sg, mh)
                                    c = seen.get(k, 0)
                                    seen[k] = c + 1
                                    lhsT = wbuf[
                                        kh * 64:(kh + 1) * 64,
                                        slot * 64:(slot + 1) * 64,
                                    ]
                                    t0 = 4 * sg + toff
                                    rhs = xv[
                                        kh * 64:(kh + 1) * 64, t0:t0 + 4,
                                        v:v + 128,
                                    ]
                                    nc.tensor.matmul(
                                        ptiles[sg % 8][mh * 64:(mh + 1) * 64],
                                        lhsT, rhs,
                                        start=(c == 0), stop=(c + 1 == nmm[k]),
                                        tile_position=(kh * 64, mh * 64),
                                    )
                    for sg in range(half * 8, half * 8 + 8):
                        st = stage_pool.tile(
                            [128, 4, 128], mybir.dt.float32, tag="st",
                            name=f"st_{_rep}_{half}_{sg}",
                        )
                        nc.vector.tensor_copy(st[:], ptiles[sg % 8][:])
                        for i in (0, 1):
                            nc.sync.dma_start(
                                out=out_ap[sg][i], in_=st[i * 64:(i + 1) * 64]
                            )

    nc.compile()
    return nc


_CACHE = {}


def _get_nc(blocks, dtag, repeat=1, plan=1):
    key = (repr(blocks), dtag, repeat, plan)
    if key not in _CACHE:
        _CACHE[key] = build(blocks, dtag, repeat, plan)
    return _CACHE[key]


def kernel(x, weights, tap_offsets):
    x = np.asarray(x)
    weights = np.asarray(weights)
    tap_offsets = np.asarray(tap_offsets)
    assert x.shape == (B, CIN, H, W)

    Weff = fold_weights(weights, tap_offsets)
    if PLAN == 1:
        blocks, Wh = make_blocks(Weff)
    else:
        blocks, Wh = make_quadrant_plan(Weff)
    np_dt = _NP_DT[DTYPE_TAG]
    nc = _get_nc(blocks, DTYPE_TAG, plan=PLAN)

    Whc = np.ascontiguousarray(Wh.astype(np_dt))
    in_maps = [
        {"xs": np.ascontiguousarray(x[b].astype(np_dt)), "wb": Whc}
        for b in range(B)
    ]
    res = run_bass_kernel_spmd(nc, in_maps, list(range(NCORES)), trace=False)
    outs = [res.results[c]["out"] for c in range(NCORES)]
    return np.stack(outs, 0).astype(np.float32)
